# revision 13
# baseline (speedup 1.0000x reference)
"""Trainium2 Bass kernel for BilinearDiscriminator.

Computes sigmoid((x*mask_x) @ W.T @ (y*mask_y).T) for x,y [8192,512],
W [512,512] -> out [8192,8192] fp32, SPMD across 8 NeuronCores.

Sharding: 8x1 (n only). Core c handles rows [c*1024, (c+1)*1024) of x and
the full 8192 columns of y, so the x@W.T transform (mm1) is computed
exactly once per row across the machine (no duplication).

All operands are fp16 on device (validated: rel err ~6e-4 vs the 2e-2
gate; fp16's 10 mantissa bits keep the logit error ~0.016 std against
logits of std ~28). Inputs are passed transposed (d-major) so both
matmuls contract over the SBUF partition dim without on-chip transposes:
  mm1: xtT[k,n] = sum_d WT[d,k]*xdT[d,n]  (PE, 4 d-chunk accumulation)
  mm2: logits[n,m] = sum_k xtT[k,n]*ydT[k,m] -> sigmoid -> out (PE+ACT)

Schedule notes (tuned against the concourse TimelineSim cost model):
- DMA issue is spread across sequencers (SP: x/y data, ACT: W, Pool:
  mask-accum + stores) and batched via strided DRAM views, since HWDGE,
  DMA_ENGINES and each SEQ are serial resources.
- Dropout masks ship as uint8 {0,1} and are applied by SWDGE accum-mult
  DMAs (dst *= src) on the Pool engine, freeing the DVE; the (1/keep)^2
  scale is folded into the sigmoid's scale operand.
- The PE clock p-state ramp (0.65/1.2/2.4 GHz, full speed only after
  3us of continuous execution) is hidden by a train of cheap warm-up
  matmuls emitted before/between the real work so the tensor engine
  never goes idle (idle resets the ramp).
- y streams in column chunks (first chunks narrow so mm2 can start
  early); output is stored f16 in 4-row-block groups, with the last
  block split off small to shorten the post-matmul tail.
"""

import os
import sys

sys.path.insert(0, "/opt/trn_rl_repo")

import numpy as np

import concourse.bass as bass
import concourse.mybir as mybir
import concourse.tile as tile
from concourse import bacc
from concourse.bass_utils import run_bass_kernel_spmd

P = 128
N, M, D = 8192, 8192, 512
NCORES = 8
N_LOC = N // NCORES  # 1024 rows of x per core
M_LOC = M  # full y per core
DC = D // P  # 4 chunks of the contraction dims
NT = N_LOC // 512  # 2 column chunks of x
NCH = N_LOC // P  # 8 output row blocks

F32 = mybir.dt.float32
F16 = mybir.dt.float16
BF16 = mybir.dt.bfloat16
U8 = mybir.dt.uint8

# y / mm2 column chunking: first chunks narrow so mm2 can start as soon
# as possible after mm1; must sum to M.
_CHUNKS = [int(w) for w in os.environ.get(
    "Y_CHUNKS", "512,512,1024,1024,1024,1024,1024,1024,512,512").split(",")]
assert sum(_CHUNKS) == M

# PE warm-up dummies (bf16 [128x128]@[128x64], ~27-53ns each): N_WARM
# before the first real matmul, N_BR1/N_BR2 bridging the mm1-nt0 -> nt1
# and nt1 -> mm2 handoffs where operand arrival can lag the PE.
N_WARM = int(os.environ.get("N_WARM", "56"))
N_BR1 = int(os.environ.get("N_BR1", "8"))
N_BR2 = int(os.environ.get("N_BR2", "8"))
OUTP_BUFS = int(os.environ.get("OUTP_BUFS", "3"))


def _build(mask_u8: bool = True):
    """Build the SPMD program.

    mask_u8=True: masks are uint8 {0,1}; the dropout scale (product of
    both masks' nonzero values) is applied via the sigmoid's scale input
    "sc". mask_u8=False: masks are arbitrary values cast to fp16 and
    applied directly; host passes sc=1.
    """
    nc = bacc.Bacc("TRN2", target_bir_lowering=False, debug=False)

    MDT = U8 if mask_u8 else F16

    xT = nc.dram_tensor("xT", [D, N_LOC], F16, kind="ExternalInput").ap()
    mxT = nc.dram_tensor("mxT", [D, N_LOC], MDT, kind="ExternalInput").ap()
    yT = nc.dram_tensor("yT", [D, M_LOC], F16, kind="ExternalInput").ap()
    myT = nc.dram_tensor("myT", [D, M_LOC], MDT, kind="ExternalInput").ap()
    wT = nc.dram_tensor("wT", [D, D], F16, kind="ExternalInput").ap()
    sc = nc.dram_tensor("sc", [P, 1], F32, kind="ExternalInput").ap()
    out = nc.dram_tensor("out", [N_LOC, M_LOC], F16, kind="ExternalOutput").ap()

    # d-major DRAM views with the 128-partition dim innermost-first
    xTv = xT.rearrange("(c p) n -> p c n", p=P)
    mxTv = mxT.rearrange("(c p) n -> p c n", p=P)
    yTv = yT.rearrange("(c p) n -> p c n", p=P)
    myTv = myT.rearrange("(c p) n -> p c n", p=P)
    wTv = wT.rearrange("(c p) k -> p c k", p=P)

    with tile.TileContext(nc) as tc:
        with (
            tc.tile_pool(name="const", bufs=1) as const_pool,
            tc.tile_pool(name="persist", bufs=1) as persist,
            tc.tile_pool(name="stage", bufs=4) as stage,
            tc.tile_pool(name="ypool", bufs=2) as ypool,
            tc.tile_pool(name="outp", bufs=OUTP_BUFS) as outp,
        ):
            sct = const_pool.tile([P, 1], F32, name="sct")
            warm = const_pool.tile([P, 192], BF16, name="warm")

            wt = persist.tile([P, DC, D], F16, name="wt")
            xdt = persist.tile([P, DC, N_LOC], F16, name="xdt")
            xtt = persist.tile([P, DC, N_LOC], F16, name="xtt")

            # PSUM: 2 x [128,512] (mm1, 2 banks) + 3 x [128,1024] (mm2,
            # 6 banks) = all 8 banks.
            psum1_ctx = tc.tile_pool(name="psum1", bufs=2, space="PSUM")
            psum1 = psum1_ctx.__enter__()
            psum2_ctx = tc.tile_pool(name="psum2", bufs=3, space="PSUM")
            psum2 = psum2_ctx.__enter__()

            nc.vector.memset(warm[:], 0.25)
            nc.sync.dma_start(out=sct[:], in_=sc[:])

            def dummies(n, tag):
                if n <= 0:
                    return
                ps = psum1.tile([P, 512], F32, name="ps1")
                for _ in range(n):
                    nc.tensor.matmul(
                        ps[:, 0:64],
                        lhsT=warm[:, 0:128],
                        rhs=warm[:, 128:192],
                        start=True,
                        stop=True,
                    )

            # ---- PE clock warm-up while the first loads are in flight
            dummies(N_WARM, "w0")

            # ---- x loads (SP, one batched DMA per nt half) + W (ACT
            # sequencer, in parallel); dropout masks applied in place on
            # the DVE per d-chunk.
            def load_x_nt(nt):
                sl = slice(nt * 512, (nt + 1) * 512)
                sm = stage.tile([P, DC, 512], MDT, name="sm", tag="sm")
                nc.sync.dma_start(out=sm[:], in_=mxTv[:, :, sl])
                nc.sync.dma_start(out=xdt[:, :, sl], in_=xTv[:, :, sl])
                return sm

            def mul_x_nt(nt, sm):
                sl = slice(nt * 512, (nt + 1) * 512)
                for dc in range(DC):
                    nc.vector.tensor_mul(
                        out=xdt[:, dc, sl], in0=xdt[:, dc, sl],
                        in1=sm[:, dc, :],
                    )

            nc.scalar.dma_start(out=wt[:], in_=wTv[:])
            sm0 = load_x_nt(0)
            sm1 = load_x_nt(1)
            mul_x_nt(0, sm0)
            mul_x_nt(1, sm1)

            # ---- y chunk loads: 2-slot rotating pool so chunk c's load
            # carries a real WAR dependency on mm2 having consumed chunk
            # c-2 — the load stream paces itself and never clogs the
            # serial DMA engine queue ahead of the mask DMAs. Masks are
            # applied in place by Pool SWDGE accum-mult DMAs (y *= m).
            coff = np.cumsum([0] + _CHUNKS)
            ytiles = {}

            def load_y(c):
                w = _CHUNKS[c]
                csl = slice(int(coff[c]), int(coff[c + 1]))
                yt = ypool.tile([P, DC, w], F16, name="ych")
                ytiles[c] = yt
                nc.sync.dma_start(out=yt[:], in_=yTv[:, :, csl])
                nc.gpsimd.dma_start(
                    out=yt[:], in_=myTv[:, :, csl],
                    accum_op=mybir.AluOpType.mult,
                )

            load_y(0)
            load_y(1)

            # ---- mm1: xtT[k,:] += WT[d,k].T @ xdT[d,:], kc-major; the
            # psum->xtt copy of block kc overlaps block kc+1 on the PE.
            def mm1_nt(nt):
                sl = slice(nt * 512, (nt + 1) * 512)
                for kc in range(DC):
                    ps = psum1.tile([P, 512], F32, name="ps1")
                    for dc in range(DC):
                        nc.tensor.matmul(
                            ps[:],
                            lhsT=wt[:, dc, kc * P : (kc + 1) * P],
                            rhs=xdt[:, dc, sl],
                            start=(dc == 0),
                            stop=(dc == DC - 1),
                        )
                    nc.vector.tensor_copy(out=xtt[:, kc, sl], in_=ps[:])

            mm1_nt(0)
            dummies(N_BR1, "b1")
            mm1_nt(1)
            dummies(N_BR2, "b2")

            # ---- mm2 + sigmoid + f16 store, chunk-major over y columns.
            # Output rows group 4 blocks per store DMA; the mask-accum
            # for chunk c+1 is emitted ahead of chunk c's stores so the
            # in-order Pool sequencer never holds stores hostage.
            def mm2_block(c, nchunk, sig, g):
                w = _CHUNKS[c]
                yt = ytiles[c]
                ps = psum2.tile([P, w], F32, name="ps2")
                for kc in range(DC):
                    for mt in range(w // 512):
                        nc.tensor.matmul(
                            ps[:, mt * 512 : (mt + 1) * 512],
                            lhsT=xtt[:, kc, nchunk * P : (nchunk + 1) * P],
                            rhs=yt[:, kc, mt * 512 : (mt + 1) * 512],
                            start=(kc == 0),
                            stop=(kc == DC - 1),
                        )
                nc.scalar.activation(
                    sig[:, g, :], ps[:], mybir.ActivationFunctionType.Sigmoid,
                    scale=sct[:],
                )

            def store_group(c, n0, ngrp, sig, engine=None):
                w = _CHUNKS[c]
                c0 = int(coff[c])
                dst = out[n0 * P : (n0 + ngrp) * P, c0 : c0 + w].rearrange(
                    "(g p) m -> p g m", p=P
                )
                (engine or nc.gpsimd).dma_start(out=dst, in_=sig[:, 0:ngrp, :])

            NCHUNKS = len(_CHUNKS)
            for c in range(NCHUNKS):
                w = _CHUNKS[c]
                last_c = c == NCHUNKS - 1
                for half in range(2):
                    n0 = half * 4
                    if half == 1 and 2 <= c + 2 < NCHUNKS:
                        load_y(c + 2)
                    if last_c and half == 1:
                        # tail: 3-block group, then a lone small store on
                        # the (idle) SP HWDGE path
                        sig = outp.tile([P, 3, w], F16, name="sig3")
                        for g in range(3):
                            mm2_block(c, n0 + g, sig, g)
                        store_group(c, n0, 3, sig)
                        sigl = outp.tile([P, 1, w], F16, name="sigl")
                        mm2_block(c, 7, sigl, 0)
                        store_group(c, 7, 1, sigl, engine=nc.sync)
                    else:
                        sig = outp.tile([P, 4, w], F16, name="sig4")
                        for g in range(4):
                            mm2_block(c, n0 + g, sig, g)
                        store_group(c, n0, 4, sig)

            psum2_ctx.__exit__(None, None, None)
            psum1_ctx.__exit__(None, None, None)

    nc.compile()
    return nc


_NC = {}


def _get_nc(mask_u8: bool = True):
    if mask_u8 not in _NC:
        _NC[mask_u8] = _build(mask_u8)
    return _NC[mask_u8]


def _two_valued(mask):
    """(is_two_valued {0, c}, c) — True for inverted-dropout masks."""
    c = float(mask.max())
    ok = bool(np.all((mask == 0) | (mask == np.float32(c))))
    return ok, c


def kernel(x, y, mask_x, mask_y, W):
    x = np.asarray(x, dtype=np.float32)
    y = np.asarray(y, dtype=np.float32)
    mask_x = np.asarray(mask_x, dtype=np.float32)
    mask_y = np.asarray(mask_y, dtype=np.float32)
    W = np.asarray(W, dtype=np.float32)

    okx, cx = _two_valued(mask_x)
    oky, cy = _two_valued(mask_y)
    mask_u8 = okx and oky
    if mask_u8:
        mxT = np.ascontiguousarray((mask_x.T != 0).astype(np.uint8))
        myT = np.ascontiguousarray((mask_y.T != 0).astype(np.uint8))
        scale = np.float32(cx) * np.float32(cy)
    else:
        mxT = np.ascontiguousarray(mask_x.T.astype(np.float16))
        myT = np.ascontiguousarray(mask_y.T.astype(np.float16))
        scale = np.float32(1.0)
    sc = np.full((P, 1), scale, dtype=np.float32)

    xT = np.ascontiguousarray(x.T.astype(np.float16))
    yT = np.ascontiguousarray(y.T.astype(np.float16))
    wT = np.ascontiguousarray(W.T.astype(np.float16))

    in_maps = []
    for c in range(NCORES):
        in_maps.append(
            {
                "xT": np.ascontiguousarray(xT[:, c * N_LOC : (c + 1) * N_LOC]),
                "mxT": np.ascontiguousarray(mxT[:, c * N_LOC : (c + 1) * N_LOC]),
                "yT": yT,
                "myT": myT,
                "wT": wT,
                "sc": sc,
            }
        )

    res = run_bass_kernel_spmd(_get_nc(mask_u8), in_maps, list(range(NCORES)))

    out = np.empty((N, M), dtype=np.float32)
    for c in range(NCORES):
        out[c * N_LOC : (c + 1) * N_LOC, :] = res.results[c]["out"].astype(
            np.float32
        )
    return out


# revision 20
# speedup vs baseline: 1.0081x; 1.0081x over previous
"""Trainium2 Bass kernel for BilinearDiscriminator.

Computes sigmoid((x*mask_x) @ W.T @ (y*mask_y).T) for x,y [8192,512],
W [512,512] -> out [8192,8192] fp32, SPMD across 8 NeuronCores.

Sharding: 8x1 (n only). Core c handles rows [c*1024, (c+1)*1024) of x and
the full 8192 columns of y, so the x@W.T transform (mm1) is computed
exactly once per row across the machine (no duplication).

All operands are fp16 on device (validated: rel err ~6e-4 vs the 2e-2
gate; fp16's 10 mantissa bits keep the logit error ~0.016 std against
logits of std ~28). Inputs are passed transposed (d-major) so both
matmuls contract over the SBUF partition dim without on-chip transposes:
  mm1: xtT[k,n] = sum_d WT[d,k]*xdT[d,n]  (PE, 4 d-chunk accumulation)
  mm2: logits[n,m] = sum_k xtT[k,n]*ydT[k,m] -> sigmoid -> out (PE+ACT)

Schedule notes (tuned against the concourse TimelineSim cost model):
- DMA issue is spread across sequencers (SP: x/y data, ACT: W, Pool:
  mask-accum + stores) and batched via strided DRAM views, since HWDGE,
  DMA_ENGINES and each SEQ are serial resources.
- Dropout masks ship as uint8 {0,1} and are applied by SWDGE accum-mult
  DMAs (dst *= src) on the Pool engine, freeing the DVE; the (1/keep)^2
  scale is folded into the sigmoid's scale operand.
- The PE clock p-state ramp (0.65/1.2/2.4 GHz, full speed only after
  3us of continuous execution) is hidden by a train of cheap warm-up
  matmuls emitted before/between the real work so the tensor engine
  never goes idle (idle resets the ramp).
- y streams in column chunks (first chunks narrow so mm2 can start
  early); output is stored f16 in 4-row-block groups, with the last
  block split off small to shorten the post-matmul tail.
"""

import os
import sys

sys.path.insert(0, "/opt/trn_rl_repo")

import numpy as np

import concourse.bass as bass
import concourse.mybir as mybir
import concourse.tile as tile
from concourse import bacc
from concourse.bass_utils import run_bass_kernel_spmd

P = 128
N, M, D = 8192, 8192, 512
NCORES = 8
N_LOC = N // NCORES  # 1024 rows of x per core
M_LOC = M  # full y per core
DC = D // P  # 4 chunks of the contraction dims
NT = N_LOC // 512  # 2 column chunks of x
NCH = N_LOC // P  # 8 output row blocks

F32 = mybir.dt.float32
F16 = mybir.dt.float16
BF16 = mybir.dt.bfloat16
U8 = mybir.dt.uint8

# y / mm2 column chunking: first chunks narrow so mm2 can start as soon
# as possible after mm1; must sum to M.
_CHUNKS = [int(w) for w in os.environ.get(
    "Y_CHUNKS", "512,512,1024,1024,1024,1024,1024,1024,1024").split(",")]
assert sum(_CHUNKS) == M

# PE warm-up dummies (bf16 [128x128]@[128x64], ~27-53ns each): N_WARM
# before the first real matmul, N_BR1/N_BR2 bridging the mm1-nt0 -> nt1
# and nt1 -> mm2 handoffs where operand arrival can lag the PE.
N_WARM = int(os.environ.get("N_WARM", "140"))
N_BR1 = int(os.environ.get("N_BR1", "8"))
N_BR2 = int(os.environ.get("N_BR2", "8"))
OUTP_BUFS = int(os.environ.get("OUTP_BUFS", "5"))


def _build(mask_u8: bool = True):
    """Build the SPMD program.

    mask_u8=True: masks are uint8 {0,1}; the dropout scale (product of
    both masks' nonzero values) is applied via the sigmoid's scale input
    "sc". mask_u8=False: masks are arbitrary values cast to fp16 and
    applied directly; host passes sc=1.
    """
    nc = bacc.Bacc("TRN2", target_bir_lowering=False, debug=False)

    MDT = U8 if mask_u8 else F16

    xT = nc.dram_tensor("xT", [D, N_LOC], F16, kind="ExternalInput").ap()
    mxT = nc.dram_tensor("mxT", [D, N_LOC], MDT, kind="ExternalInput").ap()
    yT = nc.dram_tensor("yT", [D, M_LOC], F16, kind="ExternalInput").ap()
    myT = nc.dram_tensor("myT", [D, M_LOC], MDT, kind="ExternalInput").ap()
    wT = nc.dram_tensor("wT", [D, D], F16, kind="ExternalInput").ap()
    sc = nc.dram_tensor("sc", [P, 1], F32, kind="ExternalInput").ap()
    out = nc.dram_tensor("out", [N_LOC, M_LOC], F16, kind="ExternalOutput").ap()

    # d-major DRAM views with the 128-partition dim innermost-first
    xTv = xT.rearrange("(c p) n -> p c n", p=P)
    mxTv = mxT.rearrange("(c p) n -> p c n", p=P)
    yTv = yT.rearrange("(c p) n -> p c n", p=P)
    myTv = myT.rearrange("(c p) n -> p c n", p=P)
    wTv = wT.rearrange("(c p) k -> p c k", p=P)

    with tile.TileContext(nc) as tc:
        with (
            tc.tile_pool(name="const", bufs=1) as const_pool,
            tc.tile_pool(name="persist", bufs=1) as persist,
            tc.tile_pool(name="stage", bufs=4) as stage,
            tc.tile_pool(name="ypool", bufs=2) as ypool,
            tc.tile_pool(name="outp", bufs=OUTP_BUFS) as outp,
        ):
            sct = const_pool.tile([P, 1], F32, name="sct")
            warm = const_pool.tile([P, 192], BF16, name="warm")

            wt = persist.tile([P, DC, D], F16, name="wt")
            xdt = persist.tile([P, DC, N_LOC], F16, name="xdt")
            xtt = persist.tile([P, DC, N_LOC], F16, name="xtt")

            # PSUM: 2 x [128,512] (mm1, 2 banks) + 3 x [128,1024] (mm2,
            # 6 banks) = all 8 banks.
            psum1_ctx = tc.tile_pool(name="psum1", bufs=2, space="PSUM")
            psum1 = psum1_ctx.__enter__()
            psum2_ctx = tc.tile_pool(name="psum2", bufs=3, space="PSUM")
            psum2 = psum2_ctx.__enter__()

            nc.vector.memset(warm[:], 0.25)
            nc.scalar.dma_start(out=sct[:], in_=sc[:])

            def dummies(n, tag):
                if n <= 0:
                    return
                ps = psum1.tile([P, 512], F32, name="ps1")
                for _ in range(n):
                    nc.tensor.matmul(
                        ps[:, 0:64],
                        lhsT=warm[:, 0:128],
                        rhs=warm[:, 128:192],
                        start=True,
                        stop=True,
                    )

            # ---- PE clock warm-up while the first loads are in flight
            dummies(N_WARM, "w0")

            # ---- W + x loads (SP, batched DMAs, W first since the
            # first mm1 matmul needs it); dropout masks applied in place
            # on the DVE per d-chunk.
            def load_x_nt(nt):
                sl = slice(nt * 512, (nt + 1) * 512)
                sm = stage.tile([P, DC, 512], MDT, name="sm", tag="sm")
                nc.sync.dma_start(out=sm[:], in_=mxTv[:, :, sl])
                nc.sync.dma_start(out=xdt[:, :, sl], in_=xTv[:, :, sl])
                return sm

            def mul_x_nt(nt, sm):
                sl = slice(nt * 512, (nt + 1) * 512)
                for dc in range(DC):
                    nc.vector.tensor_mul(
                        out=xdt[:, dc, sl], in0=xdt[:, dc, sl],
                        in1=sm[:, dc, :],
                    )

            nc.sync.dma_start(out=wt[:], in_=wTv[:])
            sm0 = load_x_nt(0)
            sm1 = load_x_nt(1)
            mul_x_nt(0, sm0)
            mul_x_nt(1, sm1)

            # ---- y chunk loads: 2-slot rotating pool so chunk c's load
            # carries a real WAR dependency on mm2 having consumed chunk
            # c-2. The first two chunks load via SP; later chunks load
            # via the in-order Pool sequencer, which paces them behind
            # the mask DMAs and earlier stores so they can never clog
            # the serial DMA engine queue ahead of more urgent work.
            # Masks are applied in place by Pool SWDGE accum-mult DMAs.
            coff = np.cumsum([0] + _CHUNKS)
            ytiles = {}

            def load_y(c, engine):
                w = _CHUNKS[c]
                csl = slice(int(coff[c]), int(coff[c + 1]))
                yt = ypool.tile([P, DC, w], F16, name="ych")
                ytiles[c] = yt
                engine.dma_start(out=yt[:], in_=yTv[:, :, csl])
                nc.gpsimd.dma_start(
                    out=yt[:], in_=myTv[:, :, csl],
                    accum_op=mybir.AluOpType.mult,
                )

            load_y(0, nc.sync)
            load_y(1, nc.sync)

            # ---- mm1: xtT[k,:] += WT[d,k].T @ xdT[d,:], kc-major; the
            # psum->xtt copy of block kc overlaps block kc+1 on the PE.
            def mm1_nt(nt):
                sl = slice(nt * 512, (nt + 1) * 512)
                for kc in range(DC):
                    ps = psum1.tile([P, 512], F32, name="ps1")
                    for dc in range(DC):
                        nc.tensor.matmul(
                            ps[:],
                            lhsT=wt[:, dc, kc * P : (kc + 1) * P],
                            rhs=xdt[:, dc, sl],
                            start=(dc == 0),
                            stop=(dc == DC - 1),
                        )
                    nc.vector.tensor_copy(out=xtt[:, kc, sl], in_=ps[:])

            mm1_nt(0)
            dummies(N_BR1, "b1")
            mm1_nt(1)
            dummies(N_BR2, "b2")

            # ---- mm2 + sigmoid + f16 store, chunk-major over y columns.
            # Output rows group 4 blocks per store DMA; the mask-accum
            # for chunk c+1 is emitted ahead of chunk c's stores so the
            # in-order Pool sequencer never holds stores hostage.
            def mm2_block(c, nchunk, sig, g):
                w = _CHUNKS[c]
                yt = ytiles[c]
                ps = psum2.tile([P, w], F32, name="ps2")
                for kc in range(DC):
                    for mt in range(w // 512):
                        nc.tensor.matmul(
                            ps[:, mt * 512 : (mt + 1) * 512],
                            lhsT=xtt[:, kc, nchunk * P : (nchunk + 1) * P],
                            rhs=yt[:, kc, mt * 512 : (mt + 1) * 512],
                            start=(kc == 0),
                            stop=(kc == DC - 1),
                        )
                nc.scalar.activation(
                    sig[:, g, :], ps[:], mybir.ActivationFunctionType.Sigmoid,
                    scale=sct[:],
                )

            def store_group(c, n0, ngrp, sig, engine=None):
                w = _CHUNKS[c]
                c0 = int(coff[c])
                dst = out[n0 * P : (n0 + ngrp) * P, c0 : c0 + w].rearrange(
                    "(g p) m -> p g m", p=P
                )
                (engine or nc.gpsimd).dma_start(out=dst, in_=sig[:, 0:ngrp, :])

            NCHUNKS = len(_CHUNKS)
            for c in range(NCHUNKS):
                w = _CHUNKS[c]
                last_c = c == NCHUNKS - 1
                for half in range(2):
                    n0 = half * 4
                    if half == 1 and 2 <= c + 2 < NCHUNKS:
                        load_y(c + 2, nc.gpsimd)
                    if last_c and half == 1:
                        # tail: 3-block group, then the final row block
                        # split into two half-width psum tiles so the
                        # last act+store (on the idle SP HWDGE path) is
                        # small and the post-matmul tail is short.
                        sig = outp.tile([P, 3, w], F16, name="sig3")
                        for g in range(3):
                            mm2_block(c, n0 + g, sig, g)
                        store_group(c, n0, 3, sig)
                        yt = ytiles[c]
                        for mh in range(w // 512):
                            ps = psum2.tile([P, 512], F32, name="ps2")
                            for kc in range(DC):
                                nc.tensor.matmul(
                                    ps[:],
                                    lhsT=xtt[:, kc, 7 * P : 8 * P],
                                    rhs=yt[:, kc, mh * 512 : (mh + 1) * 512],
                                    start=(kc == 0),
                                    stop=(kc == DC - 1),
                                )
                            sigl = outp.tile([P, 512], F16, name="sigl")
                            nc.scalar.activation(
                                sigl[:], ps[:],
                                mybir.ActivationFunctionType.Sigmoid,
                                scale=sct[:],
                            )
                            nc.sync.dma_start(
                                out=out[
                                    7 * P : 8 * P,
                                    int(coff[c]) + mh * 512 :
                                    int(coff[c]) + (mh + 1) * 512,
                                ],
                                in_=sigl[:],
                            )
                    else:
                        sig = outp.tile([P, 4, w], F16, name="sig4")
                        for g in range(4):
                            mm2_block(c, n0 + g, sig, g)
                        store_group(c, n0, 4, sig)

            psum2_ctx.__exit__(None, None, None)
            psum1_ctx.__exit__(None, None, None)

    nc.compile()
    return nc


_NC = {}


def _get_nc(mask_u8: bool = True):
    if mask_u8 not in _NC:
        _NC[mask_u8] = _build(mask_u8)
    return _NC[mask_u8]


def _two_valued(mask):
    """(is_two_valued {0, c}, c) — True for inverted-dropout masks."""
    c = float(mask.max())
    ok = bool(np.all((mask == 0) | (mask == np.float32(c))))
    return ok, c


def kernel(x, y, mask_x, mask_y, W):
    x = np.asarray(x, dtype=np.float32)
    y = np.asarray(y, dtype=np.float32)
    mask_x = np.asarray(mask_x, dtype=np.float32)
    mask_y = np.asarray(mask_y, dtype=np.float32)
    W = np.asarray(W, dtype=np.float32)

    okx, cx = _two_valued(mask_x)
    oky, cy = _two_valued(mask_y)
    mask_u8 = okx and oky
    if mask_u8:
        mxT = np.ascontiguousarray((mask_x.T != 0).astype(np.uint8))
        myT = np.ascontiguousarray((mask_y.T != 0).astype(np.uint8))
        scale = np.float32(cx) * np.float32(cy)
    else:
        mxT = np.ascontiguousarray(mask_x.T.astype(np.float16))
        myT = np.ascontiguousarray(mask_y.T.astype(np.float16))
        scale = np.float32(1.0)
    sc = np.full((P, 1), scale, dtype=np.float32)

    xT = np.ascontiguousarray(x.T.astype(np.float16))
    yT = np.ascontiguousarray(y.T.astype(np.float16))
    wT = np.ascontiguousarray(W.T.astype(np.float16))

    in_maps = []
    for c in range(NCORES):
        in_maps.append(
            {
                "xT": np.ascontiguousarray(xT[:, c * N_LOC : (c + 1) * N_LOC]),
                "mxT": np.ascontiguousarray(mxT[:, c * N_LOC : (c + 1) * N_LOC]),
                "yT": yT,
                "myT": myT,
                "wT": wT,
                "sc": sc,
            }
        )

    res = run_bass_kernel_spmd(_get_nc(mask_u8), in_maps, list(range(NCORES)))

    out = np.empty((N, M), dtype=np.float32)
    for c in range(NCORES):
        out[c * N_LOC : (c + 1) * N_LOC, :] = res.results[c]["out"].astype(
            np.float32
        )
    return out


# revision 22
# speedup vs baseline: 1.1192x; 1.1102x over previous
"""Trainium2 Bass kernel for BilinearDiscriminator.

Computes sigmoid((x*mask_x) @ W.T @ (y*mask_y).T) for x,y [8192,512],
W [512,512] -> out [8192,8192] fp32, SPMD across 8 NeuronCores.

Sharding: 8x1 (n only). Core c handles rows [c*1024, (c+1)*1024) of x and
the full 8192 columns of y, so the x@W.T transform (mm1) is computed
exactly once per row across the machine (no duplication).

All operands are fp16 on device (validated: rel err ~6e-4 vs the 2e-2
gate; fp16's 10 mantissa bits keep the logit error ~0.016 std against
logits of std ~28). Inputs are passed transposed (d-major) so both
matmuls contract over the SBUF partition dim without on-chip transposes:
  mm1: xtT[k,n] = sum_d WT[d,k]*xdT[d,n]  (PE, 4 d-chunk accumulation)
  mm2: logits[n,m] = sum_k xtT[k,n]*ydT[k,m] -> sigmoid -> out (PE+ACT)

Schedule notes (tuned against the concourse TimelineSim cost model):
- DMA issue is spread across sequencers (SP: x/y data, ACT: W, Pool:
  mask-accum + stores) and batched via strided DRAM views, since HWDGE,
  DMA_ENGINES and each SEQ are serial resources.
- Dropout masks ship as uint8 {0,1} and are applied by SWDGE accum-mult
  DMAs (dst *= src) on the Pool engine, freeing the DVE; the (1/keep)^2
  scale is folded into the sigmoid's scale operand.
- The PE clock p-state ramp (0.65/1.2/2.4 GHz, full speed only after
  3us of continuous execution) is hidden by a train of cheap warm-up
  matmuls emitted before/between the real work so the tensor engine
  never goes idle (idle resets the ramp).
- y streams in column chunks (first chunks narrow so mm2 can start
  early); output is stored f16 in 4-row-block groups, with the last
  block split off small to shorten the post-matmul tail.
"""

import os
import sys

sys.path.insert(0, "/opt/trn_rl_repo")

import numpy as np

import concourse.bass as bass
import concourse.mybir as mybir
import concourse.tile as tile
from concourse import bacc
from concourse.bass_utils import run_bass_kernel_spmd

P = 128
N, M, D = 8192, 8192, 512
NCORES = 8
N_LOC = N // NCORES  # 1024 rows of x per core
M_LOC = M  # full y per core
DC = D // P  # 4 chunks of the contraction dims
NT = N_LOC // 512  # 2 column chunks of x
NCH = N_LOC // P  # 8 output row blocks

F32 = mybir.dt.float32
F16 = mybir.dt.float16
BF16 = mybir.dt.bfloat16
U8 = mybir.dt.uint8

# y / mm2 column chunking: first chunks narrow so mm2 can start as soon
# as possible after mm1; must sum to M.
_CHUNKS = [int(w) for w in os.environ.get(
    "Y_CHUNKS", "512,512,1024,1024,1024,1024,1024,1024,1024").split(",")]
assert sum(_CHUNKS) == M

# PE warm-up dummies (bf16 [128x128]@[128x64], ~27-53ns each): N_WARM
# before the first real matmul, N_BR1/N_BR2 bridging the mm1-nt0 -> nt1
# and nt1 -> mm2 handoffs where operand arrival can lag the PE.
N_WARM = int(os.environ.get("N_WARM", "140"))
N_BR1 = int(os.environ.get("N_BR1", "8"))
N_BR2 = int(os.environ.get("N_BR2", "8"))
OUTP_BUFS = int(os.environ.get("OUTP_BUFS", "5"))


def _build(mask_u8: bool = True):
    """Build the SPMD program.

    mask_u8=True: masks are uint8 {0,1}; the dropout scale (product of
    both masks' nonzero values) is applied via the sigmoid's scale input
    "sc". mask_u8=False: masks are arbitrary values cast to fp16 and
    applied directly; host passes sc=1.
    """
    nc = bacc.Bacc("TRN2", target_bir_lowering=False, debug=False)

    MDT = U8 if mask_u8 else F16

    xT = nc.dram_tensor("xT", [D, N_LOC], F16, kind="ExternalInput").ap()
    mxT = nc.dram_tensor("mxT", [D, N_LOC], MDT, kind="ExternalInput").ap()
    yT = nc.dram_tensor("yT", [D, M_LOC], F16, kind="ExternalInput").ap()
    myT = nc.dram_tensor("myT", [D, M_LOC], MDT, kind="ExternalInput").ap()
    wT = nc.dram_tensor("wT", [D, D], F16, kind="ExternalInput").ap()
    sc = nc.dram_tensor("sc", [P, 1], F32, kind="ExternalInput").ap()
    out = nc.dram_tensor("out", [N_LOC, M_LOC], F16, kind="ExternalOutput").ap()

    # d-major DRAM views with the 128-partition dim innermost-first
    xTv = xT.rearrange("(c p) n -> p c n", p=P)
    mxTv = mxT.rearrange("(c p) n -> p c n", p=P)
    yTv = yT.rearrange("(c p) n -> p c n", p=P)
    myTv = myT.rearrange("(c p) n -> p c n", p=P)
    wTv = wT.rearrange("(c p) k -> p c k", p=P)

    with tile.TileContext(nc) as tc:
        with (
            tc.tile_pool(name="const", bufs=1) as const_pool,
            tc.tile_pool(name="persist", bufs=1) as persist,
            tc.tile_pool(name="stage", bufs=4) as stage,
            tc.tile_pool(name="ypool", bufs=4) as ypool,
            tc.tile_pool(name="outp", bufs=OUTP_BUFS) as outp,
        ):
            sct = const_pool.tile([P, 1], F32, name="sct")
            warm = const_pool.tile([P, 192], BF16, name="warm")

            wt = persist.tile([P, DC, D], F16, name="wt")
            xdt = persist.tile([P, DC, N_LOC], F16, name="xdt")
            xtt = persist.tile([P, DC, N_LOC], F16, name="xtt")

            # PSUM: 2 x [128,512] (mm1, 2 banks) + 3 x [128,1024] (mm2,
            # 6 banks) = all 8 banks.
            psum1_ctx = tc.tile_pool(name="psum1", bufs=2, space="PSUM")
            psum1 = psum1_ctx.__enter__()
            psum2_ctx = tc.tile_pool(name="psum2", bufs=3, space="PSUM")
            psum2 = psum2_ctx.__enter__()

            nc.vector.memset(warm[:], 0.25)
            nc.scalar.dma_start(out=sct[:], in_=sc[:])

            def dummies(n, tag):
                if n <= 0:
                    return
                ps = psum1.tile([P, 512], F32, name="ps1")
                for _ in range(n):
                    nc.tensor.matmul(
                        ps[:, 0:64],
                        lhsT=warm[:, 0:128],
                        rhs=warm[:, 128:192],
                        start=True,
                        stop=True,
                    )

            # ---- PE clock warm-up while the first loads are in flight
            dummies(N_WARM, "w0")

            # ---- W + x loads (SP, batched DMAs, W first since the
            # first mm1 matmul needs it); dropout masks applied in place
            # on the DVE per d-chunk.
            def load_x_nt(nt):
                sl = slice(nt * 512, (nt + 1) * 512)
                sm = stage.tile([P, DC, 512], MDT, name="sm", tag="sm")
                nc.sync.dma_start(out=sm[:], in_=mxTv[:, :, sl])
                nc.sync.dma_start(out=xdt[:, :, sl], in_=xTv[:, :, sl])
                return sm

            def mul_x_nt(nt, sm):
                sl = slice(nt * 512, (nt + 1) * 512)
                for dc in range(DC):
                    nc.vector.tensor_mul(
                        out=xdt[:, dc, sl], in0=xdt[:, dc, sl],
                        in1=sm[:, dc, :],
                    )

            nc.sync.dma_start(out=wt[:], in_=wTv[:])
            sm0 = load_x_nt(0)
            sm1 = load_x_nt(1)
            mul_x_nt(0, sm0)
            mul_x_nt(1, sm1)

            # ---- y chunk loads: 2-slot rotating pool so chunk c's load
            # carries a real WAR dependency on mm2 having consumed chunk
            # c-2. The first two chunks load via SP; later chunks load
            # via the in-order Pool sequencer, which paces them behind
            # the mask DMAs and earlier stores so they can never clog
            # the serial DMA engine queue ahead of more urgent work.
            # Masks are applied in place by Pool SWDGE accum-mult DMAs.
            coff = np.cumsum([0] + _CHUNKS)
            ytiles = {}

            def load_y(c, engine):
                w = _CHUNKS[c]
                csl = slice(int(coff[c]), int(coff[c + 1]))
                yt = ypool.tile([P, DC, w], F16, name="ych")
                ytiles[c] = yt
                engine.dma_start(out=yt[:], in_=yTv[:, :, csl])
                nc.gpsimd.dma_start(
                    out=yt[:], in_=myTv[:, :, csl],
                    accum_op=mybir.AluOpType.mult,
                )

            load_y(0, nc.sync)
            load_y(1, nc.sync)

            # ---- mm1: xtT[k,:] += WT[d,k].T @ xdT[d,:], kc-major; the
            # psum->xtt copy of block kc runs on the (otherwise idle)
            # ACT engine, overlapping block kc+1 on the PE and staying
            # clear of the DVE mask-multiplies.
            def mm1_nt(nt):
                sl = slice(nt * 512, (nt + 1) * 512)
                for kc in range(DC):
                    ps = psum1.tile([P, 512], F32, name="ps1")
                    for dc in range(DC):
                        nc.tensor.matmul(
                            ps[:],
                            lhsT=wt[:, dc, kc * P : (kc + 1) * P],
                            rhs=xdt[:, dc, sl],
                            start=(dc == 0),
                            stop=(dc == DC - 1),
                        )
                    nc.scalar.copy(out=xtt[:, kc, sl], in_=ps[:])

            mm1_nt(0)
            dummies(N_BR1, "b1")
            mm1_nt(1)
            dummies(N_BR2, "b2")

            # ---- mm2 + sigmoid + f16 store, chunk-major over y columns.
            # Output rows group 4 blocks per store DMA; the mask-accum
            # for chunk c+1 is emitted ahead of chunk c's stores so the
            # in-order Pool sequencer never holds stores hostage.
            def mm2_block(c, nchunk, sig, g):
                w = _CHUNKS[c]
                yt = ytiles[c]
                ps = psum2.tile([P, w], F32, name="ps2")
                for kc in range(DC):
                    for mt in range(w // 512):
                        nc.tensor.matmul(
                            ps[:, mt * 512 : (mt + 1) * 512],
                            lhsT=xtt[:, kc, nchunk * P : (nchunk + 1) * P],
                            rhs=yt[:, kc, mt * 512 : (mt + 1) * 512],
                            start=(kc == 0),
                            stop=(kc == DC - 1),
                        )
                nc.scalar.activation(
                    sig[:, g, :], ps[:], mybir.ActivationFunctionType.Sigmoid,
                    scale=sct[:],
                )

            def store_group(c, n0, ngrp, sig, engine=None):
                w = _CHUNKS[c]
                c0 = int(coff[c])
                dst = out[n0 * P : (n0 + ngrp) * P, c0 : c0 + w].rearrange(
                    "(g p) m -> p g m", p=P
                )
                (engine or nc.gpsimd).dma_start(out=dst, in_=sig[:, 0:ngrp, :])

            NCHUNKS = len(_CHUNKS)
            for c in range(NCHUNKS):
                w = _CHUNKS[c]
                last_c = c == NCHUNKS - 1
                for half in range(2):
                    n0 = half * 4
                    if half == 1 and 2 <= c + 2 < NCHUNKS:
                        load_y(c + 2, nc.gpsimd)
                    if last_c and half == 1:
                        # tail: 3-block group, then the final row block
                        # split into two half-width psum tiles so the
                        # last act+store (on the idle SP HWDGE path) is
                        # small and the post-matmul tail is short.
                        sig = outp.tile([P, 3, w], F16, name="sig3")
                        for g in range(3):
                            mm2_block(c, n0 + g, sig, g)
                        store_group(c, n0, 3, sig)
                        yt = ytiles[c]
                        for mh in range(w // 512):
                            ps = psum2.tile([P, 512], F32, name="ps2")
                            for kc in range(DC):
                                nc.tensor.matmul(
                                    ps[:],
                                    lhsT=xtt[:, kc, 7 * P : 8 * P],
                                    rhs=yt[:, kc, mh * 512 : (mh + 1) * 512],
                                    start=(kc == 0),
                                    stop=(kc == DC - 1),
                                )
                            sigl = outp.tile([P, 512], F16, name="sigl")
                            nc.scalar.activation(
                                sigl[:], ps[:],
                                mybir.ActivationFunctionType.Sigmoid,
                                scale=sct[:],
                            )
                            nc.sync.dma_start(
                                out=out[
                                    7 * P : 8 * P,
                                    int(coff[c]) + mh * 512 :
                                    int(coff[c]) + (mh + 1) * 512,
                                ],
                                in_=sigl[:],
                            )
                    else:
                        sig = outp.tile([P, 4, w], F16, name="sig4")
                        for g in range(4):
                            mm2_block(c, n0 + g, sig, g)
                        store_group(c, n0, 4, sig)

            psum2_ctx.__exit__(None, None, None)
            psum1_ctx.__exit__(None, None, None)

    nc.compile()
    return nc


_NC = {}


def _get_nc(mask_u8: bool = True):
    if mask_u8 not in _NC:
        _NC[mask_u8] = _build(mask_u8)
    return _NC[mask_u8]


def _two_valued(mask):
    """(is_two_valued {0, c}, c) — True for inverted-dropout masks."""
    c = float(mask.max())
    ok = bool(np.all((mask == 0) | (mask == np.float32(c))))
    return ok, c


def kernel(x, y, mask_x, mask_y, W):
    x = np.asarray(x, dtype=np.float32)
    y = np.asarray(y, dtype=np.float32)
    mask_x = np.asarray(mask_x, dtype=np.float32)
    mask_y = np.asarray(mask_y, dtype=np.float32)
    W = np.asarray(W, dtype=np.float32)

    okx, cx = _two_valued(mask_x)
    oky, cy = _two_valued(mask_y)
    mask_u8 = okx and oky
    if mask_u8:
        mxT = np.ascontiguousarray((mask_x.T != 0).astype(np.uint8))
        myT = np.ascontiguousarray((mask_y.T != 0).astype(np.uint8))
        scale = np.float32(cx) * np.float32(cy)
    else:
        mxT = np.ascontiguousarray(mask_x.T.astype(np.float16))
        myT = np.ascontiguousarray(mask_y.T.astype(np.float16))
        scale = np.float32(1.0)
    sc = np.full((P, 1), scale, dtype=np.float32)

    xT = np.ascontiguousarray(x.T.astype(np.float16))
    yT = np.ascontiguousarray(y.T.astype(np.float16))
    wT = np.ascontiguousarray(W.T.astype(np.float16))

    in_maps = []
    for c in range(NCORES):
        in_maps.append(
            {
                "xT": np.ascontiguousarray(xT[:, c * N_LOC : (c + 1) * N_LOC]),
                "mxT": np.ascontiguousarray(mxT[:, c * N_LOC : (c + 1) * N_LOC]),
                "yT": yT,
                "myT": myT,
                "wT": wT,
                "sc": sc,
            }
        )

    res = run_bass_kernel_spmd(_get_nc(mask_u8), in_maps, list(range(NCORES)))

    out = np.empty((N, M), dtype=np.float32)
    for c in range(NCORES):
        out[c * N_LOC : (c + 1) * N_LOC, :] = res.results[c]["out"].astype(
            np.float32
        )
    return out


# revision 25
# speedup vs baseline: 1.1593x; 1.0359x over previous
"""Trainium2 Bass kernel for BilinearDiscriminator.

Computes sigmoid((x*mask_x) @ W.T @ (y*mask_y).T) for x,y [8192,512],
W [512,512] -> out [8192,8192] fp32, SPMD across 8 NeuronCores.

Sharding: 8x1 (n only). Core c handles rows [c*1024, (c+1)*1024) of x and
the full 8192 columns of y, so the x@W.T transform (mm1) is computed
exactly once per row across the machine (no duplication).

All operands are fp16 on device (validated: rel err ~6e-4 vs the 2e-2
gate; fp16's 10 mantissa bits keep the logit error ~0.016 std against
logits of std ~28). Inputs are passed transposed (d-major) so both
matmuls contract over the SBUF partition dim without on-chip transposes:
  mm1: xtT[k,n] = sum_d WT[d,k]*xdT[d,n]  (PE, 4 d-chunk accumulation)
  mm2: logits[n,m] = sum_k xtT[k,n]*ydT[k,m] -> sigmoid -> out (PE+ACT)

Schedule notes (tuned against the concourse TimelineSim cost model):
- DMA issue is spread across sequencers (SP: x/y data, ACT: W, Pool:
  mask-accum + stores) and batched via strided DRAM views, since HWDGE,
  DMA_ENGINES and each SEQ are serial resources.
- Dropout masks ship as uint8 {0,1} and are applied by SWDGE accum-mult
  DMAs (dst *= src) on the Pool engine, freeing the DVE; the (1/keep)^2
  scale is folded into the sigmoid's scale operand.
- The PE clock p-state ramp (0.65/1.2/2.4 GHz, full speed only after
  3us of continuous execution) is hidden by a train of cheap warm-up
  matmuls emitted before/between the real work so the tensor engine
  never goes idle (idle resets the ramp).
- y streams in column chunks (first chunks narrow so mm2 can start
  early); output is stored f16 in 4-row-block groups, with the last
  block split off small to shorten the post-matmul tail.
"""

import os
import sys

sys.path.insert(0, "/opt/trn_rl_repo")

import numpy as np

import concourse.bass as bass
import concourse.mybir as mybir
import concourse.tile as tile
from concourse import bacc
from concourse.bass_utils import run_bass_kernel_spmd

P = 128
N, M, D = 8192, 8192, 512
NCORES = 8
N_LOC = N // NCORES  # 1024 rows of x per core
M_LOC = M  # full y per core
DC = D // P  # 4 chunks of the contraction dims
NT = N_LOC // 512  # 2 column chunks of x
NCH = N_LOC // P  # 8 output row blocks

F32 = mybir.dt.float32
F16 = mybir.dt.float16
BF16 = mybir.dt.bfloat16
U8 = mybir.dt.uint8

# y / mm2 column chunking: first chunks narrow so mm2 can start as soon
# as possible after mm1; must sum to M.
_CHUNKS = [int(w) for w in os.environ.get(
    "Y_CHUNKS", "512,512,1024,1024,1024,1024,1024,1024,1024").split(",")]
assert sum(_CHUNKS) == M

# PE warm-up dummies (bf16 [128x128]@[128x64], ~27-53ns each): N_WARM
# before the first real matmul, N_BR1/N_BR2 bridging the mm1-nt0 -> nt1
# and nt1 -> mm2 handoffs where operand arrival can lag the PE.
N_WARM = int(os.environ.get("N_WARM", "140"))
N_BR1 = int(os.environ.get("N_BR1", "8"))
N_BR2 = int(os.environ.get("N_BR2", "8"))
OUTP_BUFS = int(os.environ.get("OUTP_BUFS", "5"))


def _build(mask_u8: bool = True):
    """Build the SPMD program.

    mask_u8=True: masks are uint8 {0,1}; the dropout scale (product of
    both masks' nonzero values) is applied via the sigmoid's scale input
    "sc". mask_u8=False: masks are arbitrary values cast to fp16 and
    applied directly; host passes sc=1.
    """
    nc = bacc.Bacc("TRN2", target_bir_lowering=False, debug=False)

    MDT = U8 if mask_u8 else F16

    xT = nc.dram_tensor("xT", [D, N_LOC], F16, kind="ExternalInput").ap()
    mxT = nc.dram_tensor("mxT", [D, N_LOC], MDT, kind="ExternalInput").ap()
    yT = nc.dram_tensor("yT", [D, M_LOC], F16, kind="ExternalInput").ap()
    myT = nc.dram_tensor("myT", [D, M_LOC], MDT, kind="ExternalInput").ap()
    wT = nc.dram_tensor("wT", [D, D], F16, kind="ExternalInput").ap()
    sc = nc.dram_tensor("sc", [P, 1], F32, kind="ExternalInput").ap()
    out = nc.dram_tensor("out", [N_LOC, M_LOC], F16, kind="ExternalOutput").ap()

    # d-major DRAM views with the 128-partition dim innermost-first
    xTv = xT.rearrange("(c p) n -> p c n", p=P)
    mxTv = mxT.rearrange("(c p) n -> p c n", p=P)
    yTv = yT.rearrange("(c p) n -> p c n", p=P)
    myTv = myT.rearrange("(c p) n -> p c n", p=P)
    wTv = wT.rearrange("(c p) k -> p c k", p=P)

    with tile.TileContext(nc) as tc:
        with (
            tc.tile_pool(name="const", bufs=1) as const_pool,
            tc.tile_pool(name="persist", bufs=1) as persist,
            tc.tile_pool(name="stage", bufs=4) as stage,
            tc.tile_pool(name="ypool", bufs=4) as ypool,
            tc.tile_pool(name="outp", bufs=OUTP_BUFS) as outp,
        ):
            sct = const_pool.tile([P, 1], F32, name="sct")
            # Warm-up operand tiles live in ypool slots 0/1 ON PURPOSE:
            # y chunks 2/3 reuse those slots, so their loads carry a WAR
            # dependency that releases exactly when the PE warm-up (w0)
            # / bridge (b1,b2) dummies finish — pacing the loads without
            # clogging the early DMA window (the greedy scheduler hoists
            # any dep-free DMA to t=0 regardless of emission order).
            warm_a = ypool.tile([P, DC, 192], F16, name="ych")
            warm_b = ypool.tile([P, DC, 192], F16, name="ych")
            warms = {"w0": warm_a, "b1": warm_b, "b2": warm_b}

            wt = persist.tile([P, DC, D], F16, name="wt")
            xdt = persist.tile([P, DC, N_LOC], F16, name="xdt")
            xtt = persist.tile([P, DC, N_LOC], F16, name="xtt")

            # PSUM: 2 x [128,512] (mm1, 2 banks) + 3 x [128,1024] (mm2,
            # 6 banks) = all 8 banks.
            psum1_ctx = tc.tile_pool(name="psum1", bufs=2, space="PSUM")
            psum1 = psum1_ctx.__enter__()
            psum2_ctx = tc.tile_pool(name="psum2", bufs=3, space="PSUM")
            psum2 = psum2_ctx.__enter__()

            nc.vector.memset(warm_a[:, 0, :], 0.25)
            nc.vector.memset(warm_b[:, 0, :], 0.25)
            nc.scalar.dma_start(out=sct[:], in_=sc[:])

            def dummies(n, tag):
                if n <= 0:
                    return
                warm = warms[tag]
                ps = psum1.tile([P, 512], F32, name="ps1")
                for _ in range(n):
                    nc.tensor.matmul(
                        ps[:, 0:64],
                        lhsT=warm[:, 0, 0:128],
                        rhs=warm[:, 0, 128:192],
                        start=True,
                        stop=True,
                    )

            # ---- PE clock warm-up while the first loads are in flight
            dummies(N_WARM, "w0")

            # ---- W + x loads (SP, batched DMAs, W first since the
            # first mm1 matmul needs it); dropout masks applied in place
            # on the DVE per d-chunk.
            def load_x_nt(nt):
                sl = slice(nt * 512, (nt + 1) * 512)
                sm = stage.tile([P, DC, 512], MDT, name="sm", tag="sm")
                nc.sync.dma_start(out=sm[:], in_=mxTv[:, :, sl])
                nc.sync.dma_start(out=xdt[:, :, sl], in_=xTv[:, :, sl])
                return sm

            def mul_x_nt(nt, sm):
                sl = slice(nt * 512, (nt + 1) * 512)
                for dc in range(DC):
                    nc.vector.tensor_mul(
                        out=xdt[:, dc, sl], in0=xdt[:, dc, sl],
                        in1=sm[:, dc, :],
                    )

            nc.sync.dma_start(out=wt[:], in_=wTv[:])
            sm0 = load_x_nt(0)
            sm1 = load_x_nt(1)
            mul_x_nt(0, sm0)
            mul_x_nt(1, sm1)

            # ---- y chunk loads: 2-slot rotating pool so chunk c's load
            # carries a real WAR dependency on mm2 having consumed chunk
            # c-2. The first two chunks load via SP; later chunks load
            # via the in-order Pool sequencer, which paces them behind
            # the mask DMAs and earlier stores so they can never clog
            # the serial DMA engine queue ahead of more urgent work.
            # Masks are applied in place by Pool SWDGE accum-mult DMAs.
            coff = np.cumsum([0] + _CHUNKS)
            ytiles = {}

            def load_y(c, engine):
                w = _CHUNKS[c]
                csl = slice(int(coff[c]), int(coff[c + 1]))
                yt = ypool.tile([P, DC, w], F16, name="ych")
                ytiles[c] = yt
                engine.dma_start(out=yt[:], in_=yTv[:, :, csl])
                nc.gpsimd.dma_start(
                    out=yt[:], in_=myTv[:, :, csl],
                    accum_op=mybir.AluOpType.mult,
                )

            load_y(0, nc.sync)
            load_y(1, nc.sync)

            # ---- mm1: xtT[k,:] += WT[d,k].T @ xdT[d,:], kc-major; the
            # psum->xtt copy of block kc runs on the (otherwise idle)
            # ACT engine, overlapping block kc+1 on the PE and staying
            # clear of the DVE mask-multiplies.
            def mm1_nt(nt):
                sl = slice(nt * 512, (nt + 1) * 512)
                for kc in range(DC):
                    ps = psum1.tile([P, 512], F32, name="ps1")
                    for dc in range(DC):
                        nc.tensor.matmul(
                            ps[:],
                            lhsT=wt[:, dc, kc * P : (kc + 1) * P],
                            rhs=xdt[:, dc, sl],
                            start=(dc == 0),
                            stop=(dc == DC - 1),
                        )
                    nc.scalar.copy(out=xtt[:, kc, sl], in_=ps[:])

            mm1_nt(0)
            dummies(N_BR1, "b1")
            mm1_nt(1)
            dummies(N_BR2, "b2")

            # ---- mm2 + sigmoid + f16 store, chunk-major over y columns.
            # Output rows group 4 blocks per store DMA; the mask-accum
            # for chunk c+1 is emitted ahead of chunk c's stores so the
            # in-order Pool sequencer never holds stores hostage.
            def mm2_block(c, nchunk, sig, g):
                w = _CHUNKS[c]
                yt = ytiles[c]
                ps = psum2.tile([P, w], F32, name="ps2")
                for kc in range(DC):
                    for mt in range(w // 512):
                        nc.tensor.matmul(
                            ps[:, mt * 512 : (mt + 1) * 512],
                            lhsT=xtt[:, kc, nchunk * P : (nchunk + 1) * P],
                            rhs=yt[:, kc, mt * 512 : (mt + 1) * 512],
                            start=(kc == 0),
                            stop=(kc == DC - 1),
                        )
                nc.scalar.activation(
                    sig[:, g, :], ps[:], mybir.ActivationFunctionType.Sigmoid,
                    scale=sct[:],
                )

            def store_group(c, n0, ngrp, sig, engine=None):
                w = _CHUNKS[c]
                c0 = int(coff[c])
                dst = out[n0 * P : (n0 + ngrp) * P, c0 : c0 + w].rearrange(
                    "(g p) m -> p g m", p=P
                )
                (engine or nc.gpsimd).dma_start(out=dst, in_=sig[:, 0:ngrp, :])

            NCHUNKS = len(_CHUNKS)
            for c in range(NCHUNKS):
                w = _CHUNKS[c]
                last_c = c == NCHUNKS - 1
                for half in range(2):
                    n0 = half * 4
                    if half == 1 and 2 <= c + 2 < NCHUNKS:
                        load_y(c + 2, nc.gpsimd)
                    if last_c and half == 1:
                        # tail: 2-block group, a single block on the SP
                        # HWDGE path, then the final row block split into
                        # two half-width psum tiles so every remaining
                        # store shrinks as the program drains.
                        sig = outp.tile([P, 2, w], F16, name="sig3")
                        for g in range(2):
                            mm2_block(c, n0 + g, sig, g)
                        store_group(c, n0, 2, sig)
                        sig6 = outp.tile([P, 1, w], F16, name="sig6")
                        mm2_block(c, 6, sig6, 0)
                        store_group(c, 6, 1, sig6, engine=nc.sync)
                        yt = ytiles[c]
                        for mh in range(w // 512):
                            ps = psum2.tile([P, 512], F32, name="ps2")
                            for kc in range(DC):
                                nc.tensor.matmul(
                                    ps[:],
                                    lhsT=xtt[:, kc, 7 * P : 8 * P],
                                    rhs=yt[:, kc, mh * 512 : (mh + 1) * 512],
                                    start=(kc == 0),
                                    stop=(kc == DC - 1),
                                )
                            sigl = outp.tile([P, 512], F16, name="sigl")
                            nc.scalar.activation(
                                sigl[:], ps[:],
                                mybir.ActivationFunctionType.Sigmoid,
                                scale=sct[:],
                            )
                            nc.sync.dma_start(
                                out=out[
                                    7 * P : 8 * P,
                                    int(coff[c]) + mh * 512 :
                                    int(coff[c]) + (mh + 1) * 512,
                                ],
                                in_=sigl[:],
                            )
                    else:
                        sig = outp.tile([P, 4, w], F16, name="sig4")
                        for g in range(4):
                            mm2_block(c, n0 + g, sig, g)
                        store_group(c, n0, 4, sig)

            psum2_ctx.__exit__(None, None, None)
            psum1_ctx.__exit__(None, None, None)

    nc.compile()
    return nc


_NC = {}


def _get_nc(mask_u8: bool = True):
    if mask_u8 not in _NC:
        _NC[mask_u8] = _build(mask_u8)
    return _NC[mask_u8]


def _two_valued(mask):
    """(is_two_valued {0, c}, c) — True for inverted-dropout masks."""
    c = float(mask.max())
    ok = bool(np.all((mask == 0) | (mask == np.float32(c))))
    return ok, c


def kernel(x, y, mask_x, mask_y, W):
    x = np.asarray(x, dtype=np.float32)
    y = np.asarray(y, dtype=np.float32)
    mask_x = np.asarray(mask_x, dtype=np.float32)
    mask_y = np.asarray(mask_y, dtype=np.float32)
    W = np.asarray(W, dtype=np.float32)

    okx, cx = _two_valued(mask_x)
    oky, cy = _two_valued(mask_y)
    mask_u8 = okx and oky
    if mask_u8:
        mxT = np.ascontiguousarray((mask_x.T != 0).astype(np.uint8))
        myT = np.ascontiguousarray((mask_y.T != 0).astype(np.uint8))
        scale = np.float32(cx) * np.float32(cy)
    else:
        mxT = np.ascontiguousarray(mask_x.T.astype(np.float16))
        myT = np.ascontiguousarray(mask_y.T.astype(np.float16))
        scale = np.float32(1.0)
    sc = np.full((P, 1), scale, dtype=np.float32)

    xT = np.ascontiguousarray(x.T.astype(np.float16))
    yT = np.ascontiguousarray(y.T.astype(np.float16))
    wT = np.ascontiguousarray(W.T.astype(np.float16))

    in_maps = []
    for c in range(NCORES):
        in_maps.append(
            {
                "xT": np.ascontiguousarray(xT[:, c * N_LOC : (c + 1) * N_LOC]),
                "mxT": np.ascontiguousarray(mxT[:, c * N_LOC : (c + 1) * N_LOC]),
                "yT": yT,
                "myT": myT,
                "wT": wT,
                "sc": sc,
            }
        )

    res = run_bass_kernel_spmd(_get_nc(mask_u8), in_maps, list(range(NCORES)))

    out = np.empty((N, M), dtype=np.float32)
    for c in range(NCORES):
        out[c * N_LOC : (c + 1) * N_LOC, :] = res.results[c]["out"].astype(
            np.float32
        )
    return out


# revision 30
# speedup vs baseline: 1.1687x; 1.0080x over previous
"""Trainium2 Bass kernel for BilinearDiscriminator.

Computes sigmoid((x*mask_x) @ W.T @ (y*mask_y).T) for x,y [8192,512],
W [512,512] -> out [8192,8192] fp32, SPMD across 8 NeuronCores.

Sharding: 8x1 (n only). Core c handles rows [c*1024, (c+1)*1024) of x and
the full 8192 columns of y, so the x@W.T transform (mm1) is computed
exactly once per row across the machine (no duplication).

All operands are fp16 on device (validated: rel err ~6e-4 vs the 2e-2
gate; fp16's 10 mantissa bits keep the logit error ~0.016 std against
logits of std ~28). Inputs are passed transposed (d-major) so both
matmuls contract over the SBUF partition dim without on-chip transposes:
  mm1: xtT[k,n] = sum_d WT[d,k]*xdT[d,n]  (PE, 4 d-chunk accumulation)
  mm2: logits[n,m] = sum_k xtT[k,n]*ydT[k,m] -> sigmoid -> out (PE+ACT)

Schedule notes (tuned against the concourse TimelineSim cost model):
- DMA issue is spread across sequencers (SP: x/y data, ACT: W, Pool:
  mask-accum + stores) and batched via strided DRAM views, since HWDGE,
  DMA_ENGINES and each SEQ are serial resources.
- Dropout masks ship as uint8 {0,1} and are applied by SWDGE accum-mult
  DMAs (dst *= src) on the Pool engine, freeing the DVE; the (1/keep)^2
  scale is folded into the sigmoid's scale operand.
- The PE clock p-state ramp (0.65/1.2/2.4 GHz, full speed only after
  3us of continuous execution) is hidden by a train of cheap warm-up
  matmuls emitted before/between the real work so the tensor engine
  never goes idle (idle resets the ramp).
- y streams in column chunks (first chunks narrow so mm2 can start
  early); output is stored f16 in 4-row-block groups, with the last
  block split off small to shorten the post-matmul tail.
"""

import os
import sys

sys.path.insert(0, "/opt/trn_rl_repo")

import numpy as np

import concourse.bass as bass
import concourse.mybir as mybir
import concourse.tile as tile
from concourse import bacc
from concourse.bass_utils import run_bass_kernel_spmd

P = 128
N, M, D = 8192, 8192, 512
NCORES = 8
N_LOC = N // NCORES  # 1024 rows of x per core
M_LOC = M  # full y per core
DC = D // P  # 4 chunks of the contraction dims
NT = N_LOC // 512  # 2 column chunks of x
NCH = N_LOC // P  # 8 output row blocks

F32 = mybir.dt.float32
F16 = mybir.dt.float16
BF16 = mybir.dt.bfloat16
U8 = mybir.dt.uint8

# y / mm2 column chunking: first chunks narrow so mm2 can start as soon
# as possible after mm1; must sum to M.
_CHUNKS = [int(w) for w in os.environ.get(
    "Y_CHUNKS", "512,512,1024,1024,1024,1024,1024,1024,1024").split(",")]
assert sum(_CHUNKS) == M

# PE warm-up dummies (bf16 [128x128]@[128x64], ~27-53ns each): N_WARM
# before the first real matmul, N_BR1/N_BR2 bridging the mm1-nt0 -> nt1
# and nt1 -> mm2 handoffs where operand arrival can lag the PE.
N_WARM = int(os.environ.get("N_WARM", "120"))
N_BR1 = int(os.environ.get("N_BR1", "8"))
N_BR2 = int(os.environ.get("N_BR2", "8"))
OUTP_BUFS = int(os.environ.get("OUTP_BUFS", "5"))


def _build(mask_u8: bool = True):
    """Build the SPMD program.

    mask_u8=True: masks are uint8 {0,1}; the dropout scale (product of
    both masks' nonzero values) is applied via the sigmoid's scale input
    "sc". mask_u8=False: masks are arbitrary values cast to fp16 and
    applied directly; host passes sc=1.
    """
    nc = bacc.Bacc("TRN2", target_bir_lowering=False, debug=False)

    MDT = U8 if mask_u8 else F16

    xT = nc.dram_tensor("xT", [D, N_LOC], F16, kind="ExternalInput").ap()
    mxT = nc.dram_tensor("mxT", [D, N_LOC], MDT, kind="ExternalInput").ap()
    yT = nc.dram_tensor("yT", [D, M_LOC], F16, kind="ExternalInput").ap()
    myT = nc.dram_tensor("myT", [D, M_LOC], MDT, kind="ExternalInput").ap()
    wT = nc.dram_tensor("wT", [D, D], F16, kind="ExternalInput").ap()
    sc = nc.dram_tensor("sc", [P, 1], F32, kind="ExternalInput").ap()
    out = nc.dram_tensor("out", [N_LOC, M_LOC], F16, kind="ExternalOutput").ap()

    # d-major DRAM views with the 128-partition dim innermost-first
    xTv = xT.rearrange("(c p) n -> p c n", p=P)
    mxTv = mxT.rearrange("(c p) n -> p c n", p=P)
    yTv = yT.rearrange("(c p) n -> p c n", p=P)
    myTv = myT.rearrange("(c p) n -> p c n", p=P)
    wTv = wT.rearrange("(c p) k -> p c k", p=P)

    with tile.TileContext(nc) as tc:
        with (
            tc.tile_pool(name="const", bufs=1) as const_pool,
            tc.tile_pool(name="persist", bufs=1) as persist,
            tc.tile_pool(name="stage", bufs=4) as stage,
            tc.tile_pool(name="ypool", bufs=3) as ypool,
            tc.tile_pool(name="outp", bufs=OUTP_BUFS) as outp,
        ):
            sct = const_pool.tile([P, 1], F32, name="sct")
            warm = const_pool.tile([P, 192], F16, name="warm")

            wt = persist.tile([P, DC, D], F16, name="wt")
            # xdt lives in a ypool slot ON PURPOSE: y chunk 2 reuses the
            # slot, so its load carries a WAR dependency that releases
            # exactly when mm1 finishes consuming x — pacing the y
            # stream without clogging the early DMA window (the greedy
            # scheduler hoists any dep-free DMA to t=0 regardless of
            # emission order).
            xdt = ypool.tile([P, DC, N_LOC], F16, name="ych")
            xtt = persist.tile([P, DC, N_LOC], F16, name="xtt")

            # PSUM: 2 x [128,512] (mm1, 2 banks) + 3 x [128,1024] (mm2,
            # 6 banks) = all 8 banks.
            psum1_ctx = tc.tile_pool(name="psum1", bufs=2, space="PSUM")
            psum1 = psum1_ctx.__enter__()
            psum2_ctx = tc.tile_pool(name="psum2", bufs=3, space="PSUM")
            psum2 = psum2_ctx.__enter__()

            nc.vector.memset(warm[:], 0.25)
            nc.scalar.dma_start(out=sct[:], in_=sc[:])

            def dummies(n, tag):
                if n <= 0:
                    return
                ps = psum1.tile([P, 512], F32, name="ps1")
                for _ in range(n):
                    nc.tensor.matmul(
                        ps[:, 0:64],
                        lhsT=warm[:, 0:128],
                        rhs=warm[:, 128:192],
                        start=True,
                        stop=True,
                    )

            # ---- PE clock warm-up while the first loads are in flight
            dummies(N_WARM, "w0")

            # ---- W + x loads (SP, batched DMAs, W first since the
            # first mm1 matmul needs it); dropout masks applied in place
            # on the DVE per d-chunk.
            def load_x_nt(nt):
                sl = slice(nt * 512, (nt + 1) * 512)
                sm = stage.tile([P, DC, 512], MDT, name="sm", tag="sm")
                nc.sync.dma_start(out=sm[:], in_=mxTv[:, :, sl])
                nc.sync.dma_start(out=xdt[:, :, sl], in_=xTv[:, :, sl])
                return sm

            def mul_x_nt(nt, sm):
                sl = slice(nt * 512, (nt + 1) * 512)
                for dc in range(DC):
                    nc.vector.tensor_mul(
                        out=xdt[:, dc, sl], in0=xdt[:, dc, sl],
                        in1=sm[:, dc, :],
                    )

            sm0 = load_x_nt(0)
            nc.sync.dma_start(out=wt[:, 0:2, :], in_=wTv[:, 0:2, :])
            nc.sync.dma_start(out=wt[:, 2:4, :], in_=wTv[:, 2:4, :])
            sm1 = load_x_nt(1)
            mul_x_nt(0, sm0)
            mul_x_nt(1, sm1)

            # ---- y chunk loads: 2-slot rotating pool so chunk c's load
            # carries a real WAR dependency on mm2 having consumed chunk
            # c-2. The first two chunks load via SP; later chunks load
            # via the in-order Pool sequencer, which paces them behind
            # the mask DMAs and earlier stores so they can never clog
            # the serial DMA engine queue ahead of more urgent work.
            # Masks are applied in place by Pool SWDGE accum-mult DMAs.
            coff = np.cumsum([0] + _CHUNKS)
            ytiles = {}

            def load_y(c, engine):
                w = _CHUNKS[c]
                csl = slice(int(coff[c]), int(coff[c + 1]))
                yt = ypool.tile([P, DC, w], F16, name="ych")
                ytiles[c] = yt
                engine.dma_start(out=yt[:], in_=yTv[:, :, csl])
                nc.gpsimd.dma_start(
                    out=yt[:], in_=myTv[:, :, csl],
                    accum_op=mybir.AluOpType.mult,
                )

            load_y(0, nc.sync)
            load_y(1, nc.sync)

            # ---- mm1: xtT[k,:] += WT[d,k].T @ xdT[d,:], kc-major; the
            # psum->xtt copy of block kc runs on the (otherwise idle)
            # ACT engine, overlapping block kc+1 on the PE and staying
            # clear of the DVE mask-multiplies.
            def mm1_nt(nt):
                sl = slice(nt * 512, (nt + 1) * 512)
                for kc in range(DC):
                    ps = psum1.tile([P, 512], F32, name="ps1")
                    for dc in range(DC):
                        nc.tensor.matmul(
                            ps[:],
                            lhsT=wt[:, dc, kc * P : (kc + 1) * P],
                            rhs=xdt[:, dc, sl],
                            start=(dc == 0),
                            stop=(dc == DC - 1),
                        )
                    nc.scalar.copy(out=xtt[:, kc, sl], in_=ps[:])

            mm1_nt(0)
            dummies(N_BR1, "b1")
            mm1_nt(1)
            dummies(N_BR2, "b2")

            # ---- mm2 + sigmoid + f16 store, chunk-major over y columns.
            # Output rows group 4 blocks per store DMA; the mask-accum
            # for chunk c+1 is emitted ahead of chunk c's stores so the
            # in-order Pool sequencer never holds stores hostage.
            def mm2_block(c, nchunk, sig, g):
                w = _CHUNKS[c]
                yt = ytiles[c]
                ps = psum2.tile([P, w], F32, name="ps2")
                for kc in range(DC):
                    for mt in range(w // 512):
                        nc.tensor.matmul(
                            ps[:, mt * 512 : (mt + 1) * 512],
                            lhsT=xtt[:, kc, nchunk * P : (nchunk + 1) * P],
                            rhs=yt[:, kc, mt * 512 : (mt + 1) * 512],
                            start=(kc == 0),
                            stop=(kc == DC - 1),
                        )
                nc.scalar.activation(
                    sig[:, g, :], ps[:], mybir.ActivationFunctionType.Sigmoid,
                    scale=sct[:],
                )

            def store_group(c, n0, ngrp, sig, engine=None):
                w = _CHUNKS[c]
                c0 = int(coff[c])
                dst = out[n0 * P : (n0 + ngrp) * P, c0 : c0 + w].rearrange(
                    "(g p) m -> p g m", p=P
                )
                (engine or nc.gpsimd).dma_start(out=dst, in_=sig[:, 0:ngrp, :])

            NCHUNKS = len(_CHUNKS)
            for c in range(NCHUNKS):
                w = _CHUNKS[c]
                last_c = c == NCHUNKS - 1
                for half in range(2):
                    n0 = half * 4
                    if half == 1 and 2 <= c + 2 < NCHUNKS:
                        load_y(c + 2, nc.gpsimd)
                    if last_c and half == 1:
                        # tail: 2-block group, a single block on the SP
                        # HWDGE path, then the final row block split into
                        # two half-width psum tiles so every remaining
                        # store shrinks as the program drains.
                        sig = outp.tile([P, 2, w], F16, name="sig3")
                        for g in range(2):
                            mm2_block(c, n0 + g, sig, g)
                        store_group(c, n0, 2, sig)
                        sig6 = outp.tile([P, 1, w], F16, name="sig6")
                        mm2_block(c, 6, sig6, 0)
                        store_group(c, 6, 1, sig6, engine=nc.sync)
                        yt = ytiles[c]
                        for mh in range(w // 512):
                            ps = psum2.tile([P, 512], F32, name="ps2")
                            for kc in range(DC):
                                nc.tensor.matmul(
                                    ps[:],
                                    lhsT=xtt[:, kc, 7 * P : 8 * P],
                                    rhs=yt[:, kc, mh * 512 : (mh + 1) * 512],
                                    start=(kc == 0),
                                    stop=(kc == DC - 1),
                                )
                            sigl = outp.tile([P, 512], F16, name="sigl")
                            nc.scalar.activation(
                                sigl[:], ps[:],
                                mybir.ActivationFunctionType.Sigmoid,
                                scale=sct[:],
                            )
                            nc.sync.dma_start(
                                out=out[
                                    7 * P : 8 * P,
                                    int(coff[c]) + mh * 512 :
                                    int(coff[c]) + (mh + 1) * 512,
                                ],
                                in_=sigl[:],
                            )
                    else:
                        sig = outp.tile([P, 4, w], F16, name="sig4")
                        for g in range(4):
                            mm2_block(c, n0 + g, sig, g)
                        store_group(c, n0, 4, sig)

            psum2_ctx.__exit__(None, None, None)
            psum1_ctx.__exit__(None, None, None)

    nc.compile()
    return nc


_NC = {}


def _get_nc(mask_u8: bool = True):
    if mask_u8 not in _NC:
        _NC[mask_u8] = _build(mask_u8)
    return _NC[mask_u8]


def _two_valued(mask):
    """(is_two_valued {0, c}, c) — True for inverted-dropout masks."""
    c = float(mask.max())
    ok = bool(np.all((mask == 0) | (mask == np.float32(c))))
    return ok, c


def kernel(x, y, mask_x, mask_y, W):
    x = np.asarray(x, dtype=np.float32)
    y = np.asarray(y, dtype=np.float32)
    mask_x = np.asarray(mask_x, dtype=np.float32)
    mask_y = np.asarray(mask_y, dtype=np.float32)
    W = np.asarray(W, dtype=np.float32)

    okx, cx = _two_valued(mask_x)
    oky, cy = _two_valued(mask_y)
    mask_u8 = okx and oky
    if mask_u8:
        mxT = np.ascontiguousarray((mask_x.T != 0).astype(np.uint8))
        myT = np.ascontiguousarray((mask_y.T != 0).astype(np.uint8))
        scale = np.float32(cx) * np.float32(cy)
    else:
        mxT = np.ascontiguousarray(mask_x.T.astype(np.float16))
        myT = np.ascontiguousarray(mask_y.T.astype(np.float16))
        scale = np.float32(1.0)
    sc = np.full((P, 1), scale, dtype=np.float32)

    xT = np.ascontiguousarray(x.T.astype(np.float16))
    yT = np.ascontiguousarray(y.T.astype(np.float16))
    wT = np.ascontiguousarray(W.T.astype(np.float16))

    in_maps = []
    for c in range(NCORES):
        in_maps.append(
            {
                "xT": np.ascontiguousarray(xT[:, c * N_LOC : (c + 1) * N_LOC]),
                "mxT": np.ascontiguousarray(mxT[:, c * N_LOC : (c + 1) * N_LOC]),
                "yT": yT,
                "myT": myT,
                "wT": wT,
                "sc": sc,
            }
        )

    res = run_bass_kernel_spmd(_get_nc(mask_u8), in_maps, list(range(NCORES)))

    out = np.empty((N, M), dtype=np.float32)
    for c in range(NCORES):
        out[c * N_LOC : (c + 1) * N_LOC, :] = res.results[c]["out"].astype(
            np.float32
        )
    return out
